# revision 12
# baseline (speedup 1.0000x reference)
"""Trainium2 Bass kernel for nn_LlamaAttention_11716670784175.

Llama-style attention over concatenated [encoder; decoder] states with
partial RoPE (decoder positions only), a structural enc/dec block mask plus a
runtime decoder-decoder mask, returning (decoder o_proj output, full softmax
attention probabilities).

Sharding: tensor-parallel over heads across 8 NeuronCores (2 heads/core).
Wq/Wk/Wv are column-sharded by head, attention + softmax run per head-shard,
ctx (attn @ V, decoder queries only) is AllGather'd on-device per batch, then
each core computes a 256-column slice of the o_proj output (Wo row-sharded).

Softmax runs without max-subtraction (scores are O(5), exp is safe in fp32)
in two layouts: natural [q, kc] for the attention-probability output, and
transposed [kc, q] (decoder queries only) to feed PV on the PE, with row sums
from the natural pass's activation accumulators.

Shapes (hardcoded): B=2, E=D=1024, S=2048, H=2048, NH=16, HD=128.
"""

from contextlib import ExitStack
from types import SimpleNamespace

import numpy as np

import concourse.bass as bass
import concourse.bacc as bacc
import concourse.mybir as mybir
import concourse.tile as tile
from concourse.bass_utils import run_bass_kernel_spmd
from concourse.masks import make_identity

import ml_dtypes

F32 = mybir.dt.float32
F16 = mybir.dt.float16
BF16 = mybir.dt.bfloat16
EXP = mybir.ActivationFunctionType.Exp
MUL = mybir.AluOpType.mult
ADD = mybir.AluOpType.add

B = 2
E = 1024          # encoder length
D = 1024          # decoder length
S = E + D         # 2048
H = 2048
NH = 16
HD = 128
NCORES = 8
NHL = NH // NCORES          # 2 local heads
SCALE = float(HD) ** -0.5


def _qkv_phase(g, b):
    """Project q/k (transposed layout) and v (via vT + PE transpose),
    then RoPE the decoder slices of q/k. Returns (qk_tiles, vn_tiles)."""
    nc = g.nc
    qk_tiles = []
    for nm in ("q0", "q1", "k0", "k1"):
        t = g.qkp.tile([128, S], F16, tag="qk", name=f"{nm}_{b}")
        qk_tiles.append(t)
    vt_tiles = []
    for h in range(NHL):
        t = g.vtp.tile([128, S], F16, tag="vt", name=f"vt{h}_{b}")
        vt_tiles.append(t)

    for tg in range(4):          # 512-token groups
        chunks = []
        for k in range(16):
            if b == 0 and tg == 0:        # interleave weight loads on startup
                wt = g.wqp.tile([128, 6 * HD], F16, tag="wqkv", name=f"wq{k}")
                nc.sync.dma_start(wt[:], g.wqkvT[k * 128:(k + 1) * 128, :])
                g.wq_tiles.append(wt)
            hst = g.hsp.tile([128, 512], F16, tag="hs", name="hst")
            nc.sync.dma_start(
                hst[:],
                g.hsT[k * 128:(k + 1) * 128,
                      b * S + tg * 512: b * S + (tg + 1) * 512],
            )
            chunks.append(hst)
        sl = slice(tg * 512, (tg + 1) * 512)
        for ot in range(6):       # k-inner: one PSUM accumulator at a time
            acc = g.ps([128, 512])
            for k in range(16):
                nc.tensor.matmul(
                    acc[:],
                    g.wq_tiles[k][:, ot * 128:(ot + 1) * 128],
                    chunks[k][:],
                    start=(k == 0), stop=(k == 15),
                )
            if ot < 4:            # q0,q1,k0,k1 drains on DVE
                nc.vector.tensor_copy(qk_tiles[ot][:, sl], acc[:])
            else:                 # vT drains on ACT
                nc.scalar.copy(vt_tiles[ot - 4][:, sl], acc[:])

    # vT -> v natural via PE transpose
    vn_tiles = {}
    for h in range(NHL):
        for kt in range(16):
            trp = g.ps16([128, 128])
            nc.tensor.transpose(
                trp[:], vt_tiles[h][:, kt * 128:(kt + 1) * 128], g.ident16[:],
            )
            vn = g.vnp.tile([128, 128], F16, tag="vn", name=f"vn{h}_{kt}")
            nc.vector.tensor_copy(vn[:], trp[:])
            vn_tiles[(h, kt)] = vn

    # RoPE on decoder slice of q/k
    cos_t = g.csp.tile([128, D], F16, tag="cos", name=f"cos{b}")
    nc.sync.dma_start(cos_t[:], g.cosT[b])
    sin_t = g.csp.tile([128, D], F16, tag="sin", name=f"sin{b}")
    nc.sync.dma_start(sin_t[:], g.sinT[b])
    # negate top half of sin (rotate_half sign trick)
    nc.vector.tensor_scalar_mul(sin_t[0:64, :], sin_t[0:64, :], -1.0)

    for t in qk_tiles:
        xd = t[:, E:S]
        xrot = g.rtmp.tile([128, D], F16, tag="rtmp", name="ropetmp")
        # rotate_half via partition-shifting SBUF->SBUF DMAs (DVE can't
        # cross partitions); sin's top half is pre-negated above.
        nc.sync.dma_start(xrot[0:64, :], xd[64:128, :])
        nc.sync.dma_start(xrot[64:128, :], xd[0:64, :])
        nc.vector.tensor_tensor(xrot[:], xrot[:], sin_t[:], MUL)
        nc.vector.tensor_tensor(xd, xd, cos_t[:], MUL)
        nc.vector.tensor_tensor(xd, xd, xrot[:], ADD)
    return qk_tiles, vn_tiles


def _natural_softmax(g, b, qk_tiles):
    """Row-softmax in natural [q, kc] layout; writes normalized attention
    probabilities (fp16) to DRAM and returns reciprocal row-sum tiles."""
    nc = g.nc
    mkN_tiles = []
    for i in range(8):
        m = g.mkp.tile([128, D], BF16, tag="mask", name=f"mn{i}")
        nc.sync.dma_start(m[:], g.maskN[b, i * 128:(i + 1) * 128, :])
        mkN_tiles.append(m)

    recips = {}
    for qt in range(16):
        for h in range(NHL):
            qT = qk_tiles[h]
            kT = qk_tiles[2 + h]
            enc_q = qt < 8
            pn0 = g.pnp.tile([128, E], F16, tag="pn", name="pn0")
            pn1 = None
            partials = []
            for j in range(2):
                sp = g.ps([128, 512])
                nc.tensor.matmul(
                    sp[:], qT[:, qt * 128:(qt + 1) * 128],
                    kT[:, j * 512:(j + 1) * 512],
                    start=True, stop=True,
                )
                a = g.smp.tile([128, 1], F32, tag="pa", name="pa")
                nc.scalar.activation(
                    pn0[:, j * 512:(j + 1) * 512], sp[:], EXP,
                    scale=SCALE, accum_out=a[:],
                )
                partials.append(a)
            if not enc_q:
                pn1 = g.pnp.tile([128, D], F16, tag="pn", name="pn1")
                for j in range(2):
                    sp = g.ps([128, 512])
                    nc.tensor.matmul(
                        sp[:], qT[:, qt * 128:(qt + 1) * 128],
                        kT[:, E + j * 512:E + (j + 1) * 512],
                        start=True, stop=True,
                    )
                    nc.vector.scalar_tensor_tensor(
                        pn1[:, j * 512:(j + 1) * 512], sp[:], SCALE,
                        mkN_tiles[qt - 8][:, j * 512:(j + 1) * 512],
                        op0=MUL, op1=ADD,
                    )
                a2 = g.smp.tile([128, 1], F32, tag="pa", name="pa2")
                nc.scalar.activation(pn1[:], pn1[:], EXP,
                                     scale=1.0, accum_out=a2[:])
                partials.append(a2)
            s = g.smp.tile([128, 1], F32, tag="sum", name="sum")
            nc.vector.tensor_tensor(s[:], partials[0][:], partials[1][:], ADD)
            if len(partials) == 3:
                nc.vector.tensor_tensor(s[:], s[:], partials[2][:], ADD)
            r = g.rcp.tile([128, 1], F32, tag="recip", name=f"r{h}_{qt}")
            nc.vector.reciprocal(r[:], s[:])
            recips[(h, qt)] = r
            nc.vector.tensor_scalar_mul(pn0[:], pn0[:], r[:])
            if enc_q:
                nc.scalar.dma_start(
                    g.attn_enc[b, h, qt * 128:(qt + 1) * 128, :], pn0[:],
                )
            else:
                nc.vector.tensor_scalar_mul(pn1[:], pn1[:], r[:])
                qd = qt - 8
                nc.scalar.dma_start(
                    g.attn_dec[b, h, qd * 128:(qd + 1) * 128, 0:E], pn0[:],
                )
                nc.scalar.dma_start(
                    g.attn_dec[b, h, qd * 128:(qd + 1) * 128, E:S], pn1[:],
                )
    return recips


def _pv_phase(g, b, qk_tiles, vn_tiles, recips):
    """Transposed-layout softmax (decoder queries only) + attn @ V,
    scaled by the reciprocal row sums; DMAs ctx shards to the AG bounce.

    qs-outer / kt-inner: one short-lived PV accumulator per (h, qs) pass so
    PSUM slots stay free for cross-phase overlap."""
    nc = g.nc
    mkT_tiles = []
    for i in range(8):
        m = g.mkp.tile([128, D], BF16, tag="mask", name=f"mt{i}")
        nc.sync.dma_start(m[:], g.maskT[b, i * 128:(i + 1) * 128, :])
        mkT_tiles.append(m)

    for qs in range(2):
        caccs = {}
        for h in range(NHL):
            qT = qk_tiles[h]
            kT = qk_tiles[2 + h]
            # reciprocal broadcast tile [128, 512] for this (h, qs)
            trp = g.ps16([1, 512])
            for j in range(4):
                qt = 8 + qs * 4 + j
                r16 = g.rtp.tile([128, 1], F16, tag="r16", name="r16")
                nc.vector.tensor_copy(r16[:], recips[(h, qt)][:])
                nc.tensor.transpose(
                    trp[0:1, j * 128:(j + 1) * 128], r16[:], g.ident16[:],
                )
            rT = g.rtp.tile([1, 512], F16, tag="rT", name="rT")
            nc.vector.tensor_copy(rT[:], trp[:])
            rb_ps = g.ps([128, 512])
            nc.tensor.matmul(rb_ps[:], g.ones_row[:], rT[:],
                             start=True, stop=True)
            rb = g.rbp.tile([128, 512], F32, tag="rb", name="rb")
            nc.vector.tensor_copy(rb[:], rb_ps[:])
            caccs[h] = (g.ps([128, 512]), rb)
        qsl = slice(E + qs * 512, E + (qs + 1) * 512)
        for kt in range(16):
            enc_k = kt < 8
            for h in range(NHL):
                qT = qk_tiles[h]
                kT = qk_tiles[2 + h]
                cacc, _ = caccs[h]
                pts = g.ptp.tile([128, 512], F16, tag="pts", name="pts")
                sp = g.ps([128, 512])
                nc.tensor.matmul(
                    sp[:], kT[:, kt * 128:(kt + 1) * 128], qT[:, qsl],
                    start=True, stop=True,
                )
                if enc_k:
                    nc.scalar.activation(pts[:], sp[:], EXP, scale=SCALE)
                else:
                    nc.vector.scalar_tensor_tensor(
                        pts[:], sp[:], SCALE,
                        mkT_tiles[kt - 8][:, qs * 512:(qs + 1) * 512],
                        op0=MUL, op1=ADD,
                    )
                    nc.scalar.activation(pts[:], pts[:], EXP, scale=1.0)
                nc.tensor.matmul(cacc[:], vn_tiles[(h, kt)][:], pts[:],
                                 start=(kt == 0), stop=(kt == 15))
        for h in range(NHL):
            cacc, rb = caccs[h]
            csb = g.ctp.tile([128, 512], F16, tag="ctx", name="ctx")
            nc.vector.tensor_tensor(csb[:], cacc[:], rb[:], MUL)
            row = h * HD
            nc.scalar.dma_start(
                g.ctx_bounce[b][row:row + 128, qs * 512:(qs + 1) * 512],
                csb[:],
            )


def _oproj_phase(g, b):
    """o_proj for batch b: out_T[b] = WoT_slice.T @ ctx_full[b] from the
    per-batch AllGather result."""
    nc = g.nc
    if not g.wo_tiles:
        for k in range(16):
            wt = g.wop.tile([128, 2 * HD], F16, tag="wo", name=f"wo{k}")
            nc.sync.dma_start(wt[:], g.woT[k * 128:(k + 1) * 128, :])
            g.wo_tiles.append(wt)
    po = [[g.ps([128, 512]) for _ in range(2)] for _ in range(2)]
    for hh in range(16):
        row = (hh // 2) * (NHL * HD) + (hh % 2) * HD
        agc = g.agp.tile([128, D], F16, tag="agc", name="agc")
        nc.sync.dma_start(agc[:], g.ag_out[b][row:row + 128, :])
        for ot in range(2):
            for ts in range(2):
                nc.tensor.matmul(
                    po[ot][ts][:],
                    g.wo_tiles[hh][:, ot * 128:(ot + 1) * 128],
                    agc[:, ts * 512:(ts + 1) * 512],
                    start=(hh == 0), stop=(hh == 15),
                )
    for ot in range(2):
        for ts in range(2):
            osb = g.osp.tile([128, 512], F32, tag="osb", name="osb")
            nc.vector.tensor_copy(osb[:], po[ot][ts][:])
            nc.scalar.dma_start(
                g.out_T[b, ot * 128:(ot + 1) * 128, ts * 512:(ts + 1) * 512],
                osb[:],
            )


def _main_phase(g):
    nc = g.nc
    g.wq_tiles = []
    g.wo_tiles = []
    pending = None
    for b in range(B):
        qk_tiles, vn_tiles = _qkv_phase(g, b)
        if pending is not None:          # o_proj(b-1) fills attention stalls
            _oproj_phase(g, pending)
        recips = _natural_softmax(g, b, qk_tiles)
        _pv_phase(g, b, qk_tiles, vn_tiles, recips)
        nc.gpsimd.collective_compute(
            "AllGather",
            mybir.AluOpType.bypass,
            replica_groups=[list(range(NCORES))],
            ins=[g.ctx_bounce[b][:]],
            outs=[g.ag_out[b][:]],
        )
        pending = b
    _oproj_phase(g, pending)


def _build_nc():
    nc = bacc.Bacc(trn_type="TRN2", target_bir_lowering=False,
                   num_devices=NCORES)
    g = SimpleNamespace(nc=nc)

    g.hsT = nc.dram_tensor("hsT", [H, B * S], F16, kind="ExternalInput")
    g.wqkvT = nc.dram_tensor("wqkvT", [H, 6 * HD], F16, kind="ExternalInput")
    g.woT = nc.dram_tensor("woT", [H, 2 * HD], F16, kind="ExternalInput")
    g.cosT = nc.dram_tensor("cosT", [B, HD, D], F16, kind="ExternalInput")
    g.sinT = nc.dram_tensor("sinT", [B, HD, D], F16, kind="ExternalInput")
    g.maskN = nc.dram_tensor("maskN", [B, D, D], BF16, kind="ExternalInput")
    g.maskT = nc.dram_tensor("maskT", [B, D, D], BF16, kind="ExternalInput")

    g.attn_enc = nc.dram_tensor("attn_enc", [B, NHL, E, E], F16,
                                kind="ExternalOutput")
    g.attn_dec = nc.dram_tensor("attn_dec", [B, NHL, D, S], F16,
                                kind="ExternalOutput")
    g.out_T = nc.dram_tensor("out_T", [B, 2 * HD, D], F32,
                             kind="ExternalOutput")

    with tile.TileContext(nc) as tc:
        with (
            tc.tile_pool(name="dram", bufs=2, space="DRAM") as dram,
            tc.tile_pool(name="psum", bufs=8, space="PSUM") as psp,
            tc.tile_pool(name="const", bufs=1) as cst,
        ):
            g.ctx_bounce = [
                dram.tile([NHL * HD, D], F16, tag="ctxb", name=f"ctxb{b}")
                for b in range(B)
            ]
            g.ag_out = [
                dram.tile([NCORES * NHL * HD, D], F16, tag="agout",
                          addr_space="Shared", name=f"agout{b}")
                for b in range(B)
            ]

            g.ident16 = cst.tile([128, 128], F16, tag="ident16")
            make_identity(nc, g.ident16[:])
            g.ones_row = cst.tile([1, 128], F16, tag="ones_row")
            nc.vector.memset(g.ones_row[:], 1.0)

            g.ps = lambda shape: psp.tile(shape, F32, tag="ps", name="pst")
            g.ps16 = lambda shape: psp.tile(shape, F16, tag="ps", name="ps16t")

            pool_specs = dict(
                wqp=16, hsp=18, qkp=8, vtp=2, vnp=48, csp=2, mkp=8,
                pnp=6, ptp=6, rtmp=2, smp=8, rcp=36, rtp=4, rbp=2,
                ctp=2, wop=16, agp=6, osp=4,
            )
            with ExitStack() as stack:
                for nm, bufs in pool_specs.items():
                    setattr(g, nm, stack.enter_context(
                        tc.tile_pool(name=nm, bufs=bufs)))
                _main_phase(g)
    return nc


_CACHED_NC = None


def _get_nc():
    global _CACHED_NC
    if _CACHED_NC is None:
        nc = _build_nc()
        nc.compile()
        _CACHED_NC = nc
    return _CACHED_NC


def _prepare_in_maps(hidden_states, encoder_states, cos, sin, attention_mask,
                     Wq, Wk, Wv, Wo):
    hs = np.concatenate([encoder_states, hidden_states], axis=1)     # [B,S,H]
    hsT = np.ascontiguousarray(
        hs.transpose(2, 0, 1).reshape(H, B * S)).astype(np.float16)
    cosT = np.ascontiguousarray(cos.transpose(0, 2, 1)).astype(np.float16)
    sinT = np.ascontiguousarray(sin.transpose(0, 2, 1)).astype(np.float16)
    mN = np.ascontiguousarray(attention_mask[:, 0]).astype(ml_dtypes.bfloat16)
    mT = np.ascontiguousarray(
        attention_mask[:, 0].transpose(0, 2, 1)).astype(ml_dtypes.bfloat16)

    in_maps = []
    for c in range(NCORES):
        g0 = NHL * c
        cols = []
        for W in (Wq, Wk, Wv):
            for h in range(NHL):
                gh = g0 + h
                cols.append(W[gh * HD:(gh + 1) * HD, :].T)           # [H,128]
        wqkvT = np.ascontiguousarray(
            np.concatenate(cols, axis=1)).astype(np.float16)
        woT = np.ascontiguousarray(
            Wo[c * 2 * HD:(c + 1) * 2 * HD, :].T).astype(np.float16)
        in_maps.append({
            "hsT": hsT,
            "wqkvT": wqkvT,
            "woT": woT,
            "cosT": cosT,
            "sinT": sinT,
            "maskN": mN,
            "maskT": mT,
        })
    return in_maps


def kernel(hidden_states, encoder_states, cos, sin, attention_mask,
           Wq, Wk, Wv, Wo, _trace=False):
    in_maps = _prepare_in_maps(hidden_states, encoder_states, cos, sin,
                               attention_mask, Wq, Wk, Wv, Wo)
    nc = _get_nc()
    res = run_bass_kernel_spmd(nc, in_maps, core_ids=list(range(NCORES)),
                               trace=_trace)
    kernel.last_results = res

    attn = np.zeros((B, NH, S, S), np.float32)
    out = np.empty((B, D, H), np.float32)
    for c in range(NCORES):
        r = res.results[c]
        g0 = NHL * c
        attn[:, g0:g0 + NHL, :E, :E] = r["attn_enc"]
        attn[:, g0:g0 + NHL, E:, :] = r["attn_dec"]
        out[:, :, c * 2 * HD:(c + 1) * 2 * HD] = r["out_T"].transpose(0, 2, 1)
    return out, attn


# revision 13
# speedup vs baseline: 1.0454x; 1.0454x over previous
"""Trainium2 Bass kernel for nn_LlamaAttention_11716670784175.

Llama-style attention over concatenated [encoder; decoder] states with
partial RoPE (decoder positions only), a structural enc/dec block mask plus a
runtime decoder-decoder mask, returning (decoder o_proj output, full softmax
attention probabilities).

Sharding: tensor-parallel over heads across 8 NeuronCores (2 heads/core).
Wq/Wk/Wv are column-sharded by head, attention + softmax run per head-shard,
ctx (attn @ V, decoder queries only) is AllGather'd on-device per batch, then
each core computes a 256-column slice of the o_proj output (Wo row-sharded).

Softmax runs without max-subtraction (scores are O(5), exp is safe in fp32)
in two layouts: natural [q, kc] for the attention-probability output, and
transposed [kc, q] (decoder queries only) to feed PV on the PE, with row sums
from the natural pass's activation accumulators.

Shapes (hardcoded): B=2, E=D=1024, S=2048, H=2048, NH=16, HD=128.
"""

from contextlib import ExitStack
from types import SimpleNamespace

import numpy as np

import concourse.bass as bass
import concourse.bacc as bacc
import concourse.mybir as mybir
import concourse.tile as tile
from concourse.bass_utils import run_bass_kernel_spmd
from concourse.masks import make_identity

import ml_dtypes

F32 = mybir.dt.float32
F16 = mybir.dt.float16
BF16 = mybir.dt.bfloat16
EXP = mybir.ActivationFunctionType.Exp
MUL = mybir.AluOpType.mult
ADD = mybir.AluOpType.add

B = 2
E = 1024          # encoder length
D = 1024          # decoder length
S = E + D         # 2048
H = 2048
NH = 16
HD = 128
NCORES = 8
NHL = NH // NCORES          # 2 local heads
SCALE = float(HD) ** -0.5


def _qkv_phase(g, b):
    """Project q/k (transposed layout) and v (via vT + PE transpose),
    then RoPE the decoder slices of q/k. Returns (qk_tiles, vn_tiles)."""
    nc = g.nc
    qk_tiles = []
    for nm in ("q0", "q1", "k0", "k1"):
        t = g.qkp.tile([128, S], F16, tag="qk", name=f"{nm}_{b}")
        qk_tiles.append(t)
    vt_tiles = []
    for h in range(NHL):
        t = g.vtp.tile([128, S], F16, tag="vt", name=f"vt{h}_{b}")
        vt_tiles.append(t)

    for tg in range(4):          # 512-token groups
        chunks = []
        for k in range(16):
            if b == 0 and tg == 0:        # interleave weight loads on startup
                wt = g.wqp.tile([128, 6 * HD], F16, tag="wqkv", name=f"wq{k}")
                nc.sync.dma_start(wt[:], g.wqkvT[k * 128:(k + 1) * 128, :])
                g.wq_tiles.append(wt)
            hst = g.hsp.tile([128, 512], F16, tag="hs", name="hst")
            nc.sync.dma_start(
                hst[:],
                g.hsT[k * 128:(k + 1) * 128,
                      b * S + tg * 512: b * S + (tg + 1) * 512],
            )
            chunks.append(hst)
        sl = slice(tg * 512, (tg + 1) * 512)
        for ot in range(6):       # k-inner: one PSUM accumulator at a time
            acc = g.psq([128, 512])
            for k in range(16):
                nc.tensor.matmul(
                    acc[:],
                    g.wq_tiles[k][:, ot * 128:(ot + 1) * 128],
                    chunks[k][:],
                    start=(k == 0), stop=(k == 15),
                )
            if ot < 4:            # q0,q1,k0,k1 drains on DVE
                nc.vector.tensor_copy(qk_tiles[ot][:, sl], acc[:])
            else:                 # vT drains on ACT
                nc.scalar.copy(vt_tiles[ot - 4][:, sl], acc[:])

    # vT -> v natural via PE transpose
    vn_tiles = {}
    for h in range(NHL):
        for kt in range(16):
            trp = g.ps16([128, 128])
            nc.tensor.transpose(
                trp[:], vt_tiles[h][:, kt * 128:(kt + 1) * 128], g.ident16[:],
            )
            vn = g.vnp.tile([128, 128], F16, tag="vn", name=f"vn{h}_{kt}")
            nc.vector.tensor_copy(vn[:], trp[:])
            vn_tiles[(h, kt)] = vn

    # RoPE on decoder slice of q/k
    cos_t = g.csp.tile([128, D], F16, tag="cos", name=f"cos{b}")
    nc.sync.dma_start(cos_t[:], g.cosT[b])
    sin_t = g.csp.tile([128, D], F16, tag="sin", name=f"sin{b}")
    nc.sync.dma_start(sin_t[:], g.sinT[b])
    # negate top half of sin (rotate_half sign trick)
    nc.vector.tensor_scalar_mul(sin_t[0:64, :], sin_t[0:64, :], -1.0)

    for t in qk_tiles:
        xd = t[:, E:S]
        xrot = g.rtmp.tile([128, D], F16, tag="rtmp", name="ropetmp")
        # rotate_half via partition-shifting SBUF->SBUF DMAs (DVE can't
        # cross partitions); sin's top half is pre-negated above.
        nc.sync.dma_start(xrot[0:64, :], xd[64:128, :])
        nc.sync.dma_start(xrot[64:128, :], xd[0:64, :])
        nc.vector.tensor_tensor(xrot[:], xrot[:], sin_t[:], MUL)
        nc.vector.tensor_tensor(xd, xd, cos_t[:], MUL)
        nc.vector.tensor_tensor(xd, xd, xrot[:], ADD)
    return qk_tiles, vn_tiles


def _natural_softmax(g, b, qk_tiles):
    """Row-softmax in natural [q, kc] layout; writes normalized attention
    probabilities (fp16) to DRAM and returns reciprocal row-sum tiles."""
    nc = g.nc
    mkN_tiles = []
    for i in range(8):
        m = g.mkp.tile([128, D], BF16, tag="mask", name=f"mn{i}")
        nc.sync.dma_start(m[:], g.maskN[b, i * 128:(i + 1) * 128, :])
        mkN_tiles.append(m)

    recips = {}
    for qt in range(16):
        for h in range(NHL):
            qT = qk_tiles[h]
            kT = qk_tiles[2 + h]
            enc_q = qt < 8
            pn0 = g.pnp.tile([128, E], F16, tag="pn", name="pn0")
            pn1 = None
            partials = []
            for j in range(2):
                sp = g.pss([128, 512])
                nc.tensor.matmul(
                    sp[:], qT[:, qt * 128:(qt + 1) * 128],
                    kT[:, j * 512:(j + 1) * 512],
                    start=True, stop=True,
                )
                a = g.smp.tile([128, 1], F32, tag="pa", name="pa")
                nc.scalar.activation(
                    pn0[:, j * 512:(j + 1) * 512], sp[:], EXP,
                    scale=SCALE, accum_out=a[:],
                )
                partials.append(a)
            if not enc_q:
                pn1 = g.pnp.tile([128, D], F16, tag="pn", name="pn1")
                for j in range(2):
                    sp = g.pss([128, 512])
                    nc.tensor.matmul(
                        sp[:], qT[:, qt * 128:(qt + 1) * 128],
                        kT[:, E + j * 512:E + (j + 1) * 512],
                        start=True, stop=True,
                    )
                    nc.vector.scalar_tensor_tensor(
                        pn1[:, j * 512:(j + 1) * 512], sp[:], SCALE,
                        mkN_tiles[qt - 8][:, j * 512:(j + 1) * 512],
                        op0=MUL, op1=ADD,
                    )
                a2 = g.smp.tile([128, 1], F32, tag="pa", name="pa2")
                nc.scalar.activation(pn1[:], pn1[:], EXP,
                                     scale=1.0, accum_out=a2[:])
                partials.append(a2)
            s = g.smp.tile([128, 1], F32, tag="sum", name="sum")
            nc.vector.tensor_tensor(s[:], partials[0][:], partials[1][:], ADD)
            if len(partials) == 3:
                nc.vector.tensor_tensor(s[:], s[:], partials[2][:], ADD)
            r = g.rcp.tile([128, 1], F32, tag="recip", name=f"r{h}_{qt}")
            nc.vector.reciprocal(r[:], s[:])
            recips[(h, qt)] = r
            nc.vector.tensor_scalar_mul(pn0[:], pn0[:], r[:])
            if enc_q:
                nc.scalar.dma_start(
                    g.attn_enc[b, h, qt * 128:(qt + 1) * 128, :], pn0[:],
                )
            else:
                nc.vector.tensor_scalar_mul(pn1[:], pn1[:], r[:])
                qd = qt - 8
                nc.scalar.dma_start(
                    g.attn_dec[b, h, qd * 128:(qd + 1) * 128, 0:E], pn0[:],
                )
                nc.scalar.dma_start(
                    g.attn_dec[b, h, qd * 128:(qd + 1) * 128, E:S], pn1[:],
                )
    return recips


def _pv_phase(g, b, qk_tiles, vn_tiles, recips):
    """Transposed-layout softmax (decoder queries only) + attn @ V,
    scaled by the reciprocal row sums; DMAs ctx shards to the AG bounce.

    qs-outer / kt-inner: one short-lived PV accumulator per (h, qs) pass so
    PSUM slots stay free for cross-phase overlap."""
    nc = g.nc
    mkT_tiles = []
    for i in range(8):
        m = g.mkp.tile([128, D], BF16, tag="mask", name=f"mt{i}")
        nc.sync.dma_start(m[:], g.maskT[b, i * 128:(i + 1) * 128, :])
        mkT_tiles.append(m)

    for qs in range(2):
        caccs = {}
        for h in range(NHL):
            qT = qk_tiles[h]
            kT = qk_tiles[2 + h]
            # reciprocal broadcast tile [128, 512] for this (h, qs)
            trp = g.ps16([1, 512])
            for j in range(4):
                qt = 8 + qs * 4 + j
                r16 = g.rtp.tile([128, 1], F16, tag="r16", name="r16")
                nc.vector.tensor_copy(r16[:], recips[(h, qt)][:])
                nc.tensor.transpose(
                    trp[0:1, j * 128:(j + 1) * 128], r16[:], g.ident16[:],
                )
            rT = g.rtp.tile([1, 512], F16, tag="rT", name="rT")
            nc.vector.tensor_copy(rT[:], trp[:])
            rb_ps = g.pss([128, 512])
            nc.tensor.matmul(rb_ps[:], g.ones_row[:], rT[:],
                             start=True, stop=True)
            rb = g.rbp.tile([128, 512], F32, tag="rb", name="rb")
            nc.vector.tensor_copy(rb[:], rb_ps[:])
            caccs[h] = (g.psc([128, 512]), rb)
        qsl = slice(E + qs * 512, E + (qs + 1) * 512)
        for kt in range(16):
            enc_k = kt < 8
            for h in range(NHL):
                qT = qk_tiles[h]
                kT = qk_tiles[2 + h]
                cacc, _ = caccs[h]
                pts = g.ptp.tile([128, 512], F16, tag="pts", name="pts")
                sp = g.pss([128, 512])
                nc.tensor.matmul(
                    sp[:], kT[:, kt * 128:(kt + 1) * 128], qT[:, qsl],
                    start=True, stop=True,
                )
                if enc_k:
                    nc.scalar.activation(pts[:], sp[:], EXP, scale=SCALE)
                else:
                    nc.vector.scalar_tensor_tensor(
                        pts[:], sp[:], SCALE,
                        mkT_tiles[kt - 8][:, qs * 512:(qs + 1) * 512],
                        op0=MUL, op1=ADD,
                    )
                    nc.scalar.activation(pts[:], pts[:], EXP, scale=1.0)
                nc.tensor.matmul(cacc[:], vn_tiles[(h, kt)][:], pts[:],
                                 start=(kt == 0), stop=(kt == 15))
        for h in range(NHL):
            cacc, rb = caccs[h]
            csb = g.ctp.tile([128, 512], F16, tag="ctx", name="ctx")
            nc.vector.tensor_tensor(csb[:], cacc[:], rb[:], MUL)
            row = h * HD
            nc.scalar.dma_start(
                g.ctx_bounce[b][row:row + 128, qs * 512:(qs + 1) * 512],
                csb[:],
            )


def _oproj_phase(g, b):
    """o_proj for batch b: out_T[b] = WoT_slice.T @ ctx_full[b] from the
    per-batch AllGather result."""
    nc = g.nc
    if not g.wo_tiles:
        for k in range(16):
            wt = g.wop.tile([128, 2 * HD], F16, tag="wo", name=f"wo{k}")
            nc.sync.dma_start(wt[:], g.woT[k * 128:(k + 1) * 128, :])
            g.wo_tiles.append(wt)
    for ot in range(2):
        po = [g.psc([128, 512]) for _ in range(2)]
        for hh in range(16):
            row = (hh // 2) * (NHL * HD) + (hh % 2) * HD
            agc = g.agp.tile([128, D], F16, tag="agc", name="agc")
            nc.sync.dma_start(agc[:], g.ag_out[b][row:row + 128, :])
            for ts in range(2):
                nc.tensor.matmul(
                    po[ts][:],
                    g.wo_tiles[hh][:, ot * 128:(ot + 1) * 128],
                    agc[:, ts * 512:(ts + 1) * 512],
                    start=(hh == 0), stop=(hh == 15),
                )
        for ts in range(2):
            osb = g.osp.tile([128, 512], F32, tag="osb", name="osb")
            nc.vector.tensor_copy(osb[:], po[ts][:])
            nc.scalar.dma_start(
                g.out_T[b, ot * 128:(ot + 1) * 128, ts * 512:(ts + 1) * 512],
                osb[:],
            )


def _main_phase(g):
    nc = g.nc
    g.wq_tiles = []
    g.wo_tiles = []
    pending = None
    for b in range(B):
        qk_tiles, vn_tiles = _qkv_phase(g, b)
        if pending is not None:          # o_proj(b-1) fills attention stalls
            _oproj_phase(g, pending)
        recips = _natural_softmax(g, b, qk_tiles)
        _pv_phase(g, b, qk_tiles, vn_tiles, recips)
        nc.gpsimd.collective_compute(
            "AllGather",
            mybir.AluOpType.bypass,
            replica_groups=[list(range(NCORES))],
            ins=[g.ctx_bounce[b][:]],
            outs=[g.ag_out[b][:]],
        )
        pending = b
    _oproj_phase(g, pending)


def _build_nc():
    nc = bacc.Bacc(trn_type="TRN2", target_bir_lowering=False,
                   num_devices=NCORES)
    g = SimpleNamespace(nc=nc)

    g.hsT = nc.dram_tensor("hsT", [H, B * S], F16, kind="ExternalInput")
    g.wqkvT = nc.dram_tensor("wqkvT", [H, 6 * HD], F16, kind="ExternalInput")
    g.woT = nc.dram_tensor("woT", [H, 2 * HD], F16, kind="ExternalInput")
    g.cosT = nc.dram_tensor("cosT", [B, HD, D], F16, kind="ExternalInput")
    g.sinT = nc.dram_tensor("sinT", [B, HD, D], F16, kind="ExternalInput")
    g.maskN = nc.dram_tensor("maskN", [B, D, D], BF16, kind="ExternalInput")
    g.maskT = nc.dram_tensor("maskT", [B, D, D], BF16, kind="ExternalInput")

    g.attn_enc = nc.dram_tensor("attn_enc", [B, NHL, E, E], F16,
                                kind="ExternalOutput")
    g.attn_dec = nc.dram_tensor("attn_dec", [B, NHL, D, S], F16,
                                kind="ExternalOutput")
    g.out_T = nc.dram_tensor("out_T", [B, 2 * HD, D], F32,
                             kind="ExternalOutput")

    with tile.TileContext(nc) as tc:
        with (
            tc.tile_pool(name="dram", bufs=2, space="DRAM") as dram,
            tc.tile_pool(name="psum", bufs=2, space="PSUM") as psp,
            tc.tile_pool(name="const", bufs=1) as cst,
        ):
            g.ctx_bounce = [
                dram.tile([NHL * HD, D], F16, tag="ctxb", name=f"ctxb{b}")
                for b in range(B)
            ]
            g.ag_out = [
                dram.tile([NCORES * NHL * HD, D], F16, tag="agout",
                          addr_space="Shared", name=f"agout{b}")
                for b in range(B)
            ]

            g.ident16 = cst.tile([128, 128], F16, tag="ident16")
            make_identity(nc, g.ident16[:])
            g.ones_row = cst.tile([1, 128], F16, tag="ones_row")
            nc.vector.memset(g.ones_row[:], 1.0)

            g.psq = lambda shape: psp.tile(shape, F32, tag="psq",
                                           name="psqt", bufs=2)
            g.pss = lambda shape: psp.tile(shape, F32, tag="pss",
                                           name="psst", bufs=4)
            g.psc = lambda shape: psp.tile(shape, F32, tag="psc",
                                           name="psct", bufs=2)
            g.ps16 = lambda shape: psp.tile(shape, F16, tag="pss",
                                            name="ps16t", bufs=4)

            pool_specs = dict(
                wqp=16, hsp=18, qkp=8, vtp=2, vnp=48, csp=2, mkp=8,
                pnp=6, ptp=6, rtmp=2, smp=8, rcp=36, rtp=4, rbp=2,
                ctp=2, wop=16, agp=6, osp=4,
            )
            with ExitStack() as stack:
                for nm, bufs in pool_specs.items():
                    setattr(g, nm, stack.enter_context(
                        tc.tile_pool(name=nm, bufs=bufs)))
                _main_phase(g)
    return nc


_CACHED_NC = None


def _get_nc():
    global _CACHED_NC
    if _CACHED_NC is None:
        nc = _build_nc()
        nc.compile()
        _CACHED_NC = nc
    return _CACHED_NC


def _prepare_in_maps(hidden_states, encoder_states, cos, sin, attention_mask,
                     Wq, Wk, Wv, Wo):
    hs = np.concatenate([encoder_states, hidden_states], axis=1)     # [B,S,H]
    hsT = np.ascontiguousarray(
        hs.transpose(2, 0, 1).reshape(H, B * S)).astype(np.float16)
    cosT = np.ascontiguousarray(cos.transpose(0, 2, 1)).astype(np.float16)
    sinT = np.ascontiguousarray(sin.transpose(0, 2, 1)).astype(np.float16)
    mN = np.ascontiguousarray(attention_mask[:, 0]).astype(ml_dtypes.bfloat16)
    mT = np.ascontiguousarray(
        attention_mask[:, 0].transpose(0, 2, 1)).astype(ml_dtypes.bfloat16)

    in_maps = []
    for c in range(NCORES):
        g0 = NHL * c
        cols = []
        for W in (Wq, Wk, Wv):
            for h in range(NHL):
                gh = g0 + h
                cols.append(W[gh * HD:(gh + 1) * HD, :].T)           # [H,128]
        wqkvT = np.ascontiguousarray(
            np.concatenate(cols, axis=1)).astype(np.float16)
        woT = np.ascontiguousarray(
            Wo[c * 2 * HD:(c + 1) * 2 * HD, :].T).astype(np.float16)
        in_maps.append({
            "hsT": hsT,
            "wqkvT": wqkvT,
            "woT": woT,
            "cosT": cosT,
            "sinT": sinT,
            "maskN": mN,
            "maskT": mT,
        })
    return in_maps


def kernel(hidden_states, encoder_states, cos, sin, attention_mask,
           Wq, Wk, Wv, Wo, _trace=False):
    in_maps = _prepare_in_maps(hidden_states, encoder_states, cos, sin,
                               attention_mask, Wq, Wk, Wv, Wo)
    nc = _get_nc()
    res = run_bass_kernel_spmd(nc, in_maps, core_ids=list(range(NCORES)),
                               trace=_trace)
    kernel.last_results = res

    attn = np.zeros((B, NH, S, S), np.float32)
    out = np.empty((B, D, H), np.float32)
    for c in range(NCORES):
        r = res.results[c]
        g0 = NHL * c
        attn[:, g0:g0 + NHL, :E, :E] = r["attn_enc"]
        attn[:, g0:g0 + NHL, E:, :] = r["attn_dec"]
        out[:, :, c * 2 * HD:(c + 1) * 2 * HD] = r["out_T"].transpose(0, 2, 1)
    return out, attn
